# revision 13
# baseline (speedup 1.0000x reference)
"""DisenGCN-style 4-layer GCN on 8 Trainium2 NeuronCores.

v11: BN-decoupled message tables.  Every layer propagates the RAW relu
output r_l (scaled by dinv); the BN affine + next-layer GEMM are applied
AFTER propagation by folding them into the weights:

    agg_{l+1} = dinv_d * (P @ diag(a_l) W_{l+1}) + w_d * (c_l W_{l+1}) + b_{l+1}
    with P[d] = sum_{e: dst=d} (dinv*r_l)[src_e] + (dinv*r_l)[d]
    and  w_d  = dinv_d * (sum_{e: dst=d} dinv_src + dinv_d)   (host precomputed)

so the next layer's table T_{l+1} = dinv * r_l is built INSIDE the
message-pass post step (no separate table_build pass, no BN wait), and the
only inter-layer serialization is the stats AllReduce + the table AllGather.
Pad nodes are corrected via the runtime recursion
    rpad_{l+1} = relu(W^T(a_l*rpad_l + c_l) + b_{l+1}),
subtracting K*rpad from the global stats.

Distribution (8 cores, SPMD single program):
  * nodes block-partitioned: core c owns rows [c*NPC, (c+1)*NPC).
  * each core handles edges whose dst it owns; tables replicated via
    AllGather after each layer; BN stats via a tiny AllReduce.
  * per-edge gather uses dma_gather (int16 indices -> table split into 4
    source buckets < 32768 rows; edges grouped by (batch of dst blocks,
    bucket, dst block), each (block,bucket) group padded to 128-edge tiles).
  * scatter-add is a one-hot matmul: per 128-edge tile, S[e, slot] =
    (dst_slot[e] == slot) built on DVE; PSUM accumulates
    acc[f, slot] += sum_e M[e, f] * S[e, slot] over the block's tiles.
"""

import os
import sys
import math
import numpy as np

sys.path.insert(0, "/opt/trn_rl_repo")

P = 128
GB = 6            # dst blocks per gather batch
BUCKET_MAX = 32768  # dma_gather int16 index limit (overridable for tests)
MP_MODE = os.environ.get("GCN_MP_MODE", "full")  # gather | smat | mm | full
F16 = bool(int(os.environ.get("GCN_F16", "1")))  # fp16 message tables
CALL_TILES = int(os.environ.get("GCN_CALL_TILES", "8"))   # tiles per dma_gather
DMA_SCRATCH = int(os.environ.get("GCN_DMA_SCRATCH", "16384"))
DEBUG_DUMP = bool(int(os.environ.get("GCN_DEBUG_DUMP", "0")))


# ---------------------------------------------------------------- host prep


def _build_plan(edge_index, N, n_cores):
    """Partition edges; build per-core index/slot streams + shared structure."""
    npc = int(math.ceil(N / n_cores / P)) * P          # nodes per core
    npad = npc * n_cores
    nb = npc // P                                       # dst blocks per core
    nbuk = max(1, int(math.ceil(npad / BUCKET_MAX)))    # source buckets
    while npad % nbuk:
        nbuk += 1
    bs = npad // nbuk
    assert bs <= BUCKET_MAX

    src0 = np.asarray(edge_index[0], dtype=np.int64)
    dst0 = np.asarray(edge_index[1], dtype=np.int64)
    # self loops are handled by an identity matmul over the block's own
    # table rows, so the edge streams carry only the directed edges.
    src = src0
    dst = dst0

    # relabel nodes so each (core, block) bin carries a near-equal edge load:
    # greedy heaviest-first assignment to the lightest non-full bin.
    import heapq
    degN = np.bincount(dst, minlength=N)
    nbins = n_cores * nb
    order = np.argsort(-degN, kind="stable")
    heap = [(0, b) for b in range(nbins)]
    heapq.heapify(heap)
    cap = np.full(nbins, P, np.int64)
    newid = np.empty(npad, np.int64)
    base = (np.arange(nbins) // nb) * npc + (np.arange(nbins) % nb) * P
    for n in order:
        while True:
            load, b = heapq.heappop(heap)
            if cap[b] > 0:
                break
        newid[n] = base[b] + (P - cap[b])
        cap[b] -= 1
        heapq.heappush(heap, (load + int(degN[n]), b))
    spots = np.concatenate([np.arange(base[b] + P - cap[b], base[b] + P)
                            for b in range(nbins)]) if cap.sum() else \
        np.empty(0, np.int64)
    newid[N:] = spots
    src = newid[src]
    dst = newid[dst]

    # degree includes the self loop
    deg = np.bincount(dst, minlength=npad).astype(np.float64)
    deg[newid[:N]] += 1.0
    dinv = np.zeros(npad, np.float32)
    nz = deg > 0
    dinv[nz] = (1.0 / np.sqrt(deg[nz])).astype(np.float32)

    # w_d = dinv_d * (sum_{e: dst=d} dinv_src + dinv_d) for the rank-1
    # BN-shift term (isolated/pad nodes: w = 1)
    q = np.bincount(dst, weights=dinv[src].astype(np.float64),
                    minlength=npad).astype(np.float32) + dinv
    w = dinv * q

    ngrp = nb * nbuk
    core_data = []
    counts = np.zeros((n_cores, ngrp), np.int64)
    for c in range(n_cores):
        m = (dst >= c * npc) & (dst < (c + 1) * npc)
        s = src[m]
        d = dst[m] - c * npc
        key = (d >> 7) * nbuk + s // bs
        order = np.argsort(key, kind="stable")
        counts[c] = np.bincount(key, minlength=ngrp)
        core_data.append((key[order], (s % bs)[order], (d & 127)[order]))

    tiles_grp = (counts.max(axis=0) + P - 1) // P       # tiles per (block,bucket)

    # stream order: for each batch of GB blocks: for bucket: for block
    n_batches = (nb + GB - 1) // GB
    grp_order = []
    batches = []    # per batch: dict(t0, nt, blks, calls=[(k,t0,nt)], bt=..)
    tpos = 0
    for g in range(n_batches):
        blks = list(range(g * GB, min((g + 1) * GB, nb)))
        b_t0 = tpos
        bcalls = []
        btiles = {b: [] for b in blks}     # (bucket, t0, nt) per block
        for k in range(nbuk):
            c_t0 = tpos
            for b in blks:
                t = int(tiles_grp[b * nbuk + k])
                if t:
                    grp_order.append(b * nbuk + k)
                    btiles[b].append((k, tpos, t))
                    tpos += t
            # dma_gather call size capped by the SWDGE ring (CALL_TILES*128
            # descriptors must fit DMA_SCRATCH//16 ring slots)
            for sub in range(c_t0, tpos, CALL_TILES):
                bcalls.append((k, sub, min(CALL_TILES, tpos - sub)))
        batches.append(dict(t0=b_t0, nt=tpos - b_t0, blks=blks,
                            calls=bcalls, btiles=btiles))
    tott = tpos
    tote = tott * P

    grp_start = np.full(ngrp, -1, np.int64)
    pos = 0
    for gid in grp_order:
        grp_start[gid] = pos
        pos += int(tiles_grp[gid]) * P

    idx_arrs, slot_arrs = [], []
    for c in range(n_cores):
        key_s, sloc_s, slot_s = core_data[c]
        ne = len(key_s)
        grp_first = np.searchsorted(key_s, np.arange(ngrp), side="left")
        within = np.arange(ne, dtype=np.int64) - grp_first[key_s]
        posi = grp_start[key_s] + within
        idx_stream = np.zeros(tote, np.int16)
        slot_stream = np.full(tote, -1.0, np.float32)
        idx_stream[posi] = sloc_s.astype(np.int16)
        slot_stream[posi] = slot_s.astype(np.float32)
        idx_arrs.append(np.ascontiguousarray(
            np.tile(idx_stream.reshape(-1, 16).T, (8, 1))))
        slot_arrs.append(np.ascontiguousarray(slot_stream.reshape(-1, P).T))

    return dict(
        N=N, n_cores=n_cores, npc=npc, npad=npad, nb=nb, nbuk=nbuk, bs=bs,
        batches=batches, tott=tott, dinv=dinv, w=w, newid=newid,
        idx_arrs=idx_arrs, slot_arrs=slot_arrs,
    )


# ------------------------------------------------------------ bass program


class _Stop(Exception):
    pass


def _build_nc(plan, stop_after=None, repeat=1):
    from concourse import bass, mybir, tile, bacc
    f32 = mybir.dt.float32
    f16 = mybir.dt.float16
    dtm = f16 if F16 else f32
    i16 = mybir.dt.int16
    Alu = mybir.AluOpType
    Act = mybir.ActivationFunctionType
    Axis = mybir.AxisListType

    npc, npad, nb, nbuk, bs = (plan[k] for k in ("npc", "npad", "nb", "nbuk", "bs"))
    tott = plan["tott"]
    n_cores = plan["n_cores"]
    N = plan["N"]
    K_pad = npad - N
    rg = [list(range(n_cores))]
    batches = plan["batches"]

    D1, D2, D3, D4 = 64, 128, 128, 64     # table content dims per layer
    F1, F2, F3, F4 = 128, 128, 64, 2      # post-conv dims
    # gather width: 128 f16 elems = 256B rows; narrow tables leave garbage
    # in the tail columns, which only ever lands in unread PSUM rows.
    DW = 128

    nc = bacc.Bacc("TRN2", target_bir_lowering=False, debug=False,
                   num_devices=n_cores, num_swdge_queues=4,
                   dynamic_dma_scratch_size=DMA_SCRATCH)

    # ---- I/O ----
    totc = tott * 8
    x_own = nc.declare_dram_parameter("x_own", [npc, 64], f32, isOutput=False)
    idx_p = nc.declare_dram_parameter("idx", [P, totc], i16, isOutput=False)
    slot_p = nc.declare_dram_parameter("slot", [P, tott], dtm, isOutput=False)
    dinvr_p = nc.declare_dram_parameter("dinvr", [P, npc], dtm, isOutput=False)
    wrow_p = nc.declare_dram_parameter("wrow", [1, npc], dtm, isOutput=False)
    dinv_nm_p = nc.declare_dram_parameter("dinv_nm", [P, nb], f32, isOutput=False)
    coliota_p = nc.declare_dram_parameter("coliota", [P, P], dtm, isOutput=False)
    ident16_p = nc.declare_dram_parameter("ident16", [P, P], dtm, isOutput=False)
    ident32_p = nc.declare_dram_parameter("ident32", [P, P], f32, isOutput=False)
    W1_p = nc.declare_dram_parameter("W1p", [64, 128], f32, isOutput=False)
    W2_p = nc.declare_dram_parameter("W2", [128, 128], f32, isOutput=False)
    W3_p = nc.declare_dram_parameter("W3", [128, 64], f32, isOutput=False)
    W4_p = nc.declare_dram_parameter("W4", [64, 2], f32, isOutput=False)
    vecs = {}
    for nm, f in (("b1", 128), ("g1", 128), ("be1", 128), ("cs1", 128),
                  ("cq1", 128), ("b2", 128), ("g2", 128), ("be2", 128),
                  ("cs2", 128), ("cq2", 128), ("b3", 64), ("g3", 64),
                  ("be3", 64), ("cs3", 64), ("cq3", 64), ("b4c", 2)):
        vecs[nm] = nc.declare_dram_parameter(nm, [f, 1], f32, isOutput=False)
    rows = {}
    for nm, f in (("b2r", 128), ("b3r", 64), ("b4r", 2)):
        rows[nm] = nc.declare_dram_parameter(nm, [1, f], f32, isOutput=False)
    out_p = nc.declare_dram_parameter("out", [P, nb, 2], f32, isOutput=True)
    dbg = {}
    if DEBUG_DUMP:
        for l in (2, 3, 4):
            dbg[f"dT{l}"] = nc.declare_dram_parameter(
                f"dT{l}", [npc, DW], dtm, isOutput=True)
        for l in (1, 2, 3):
            f = [128, 128, 64][l - 1]
            dbg[f"dst{l}"] = nc.declare_dram_parameter(
                f"dst{l}", [f, 2], f32, isOutput=True)
            dbg[f"dac{l}"] = nc.declare_dram_parameter(
                f"dac{l}", [f, 2], f32, isOutput=True)

    # ---- internal DRAM ----
    T = [None] + [nc.dram_tensor(f"T{l}", [npad, DW], dtm, addr_space="Shared")
                  for l in (1, 2, 3, 4)]
    Tc = [None] + [nc.dram_tensor(f"T{l}c", [npc, DW], dtm)
                   for l in (1, 2, 3, 4)]
    st_in = [None] + [nc.dram_tensor(f"stin{l}", [[128, 128, 64][l - 1], 2], f32)
                      for l in (1, 2, 3)]
    st_out = [None] + [nc.dram_tensor(f"stout{l}", [[128, 128, 64][l - 1], 2],
                                      f32, addr_space="Shared")
                       for l in (1, 2, 3)]

    with tile.TileContext(nc) as tc:
        with (
            tc.tile_pool(name="const", bufs=1) as cpool,
            tc.tile_pool(name="slotp", bufs=1) as slpool,
            tc.tile_pool(name="msg", bufs=max(4, 112 // CALL_TILES)) as mpool,
            tc.tile_pool(name="smat", bufs=max(2, 32 // CALL_TILES)) as spool,
            tc.tile_pool(name="blk", bufs=3) as bpool,
            tc.tile_pool(name="stats", bufs=1) as stpool,
            tc.tile_pool(name="ownp", bufs=1) as opool,
        ):
            # ---------- constants ----------
            def load_const(handle, shape):
                t = cpool.tile(shape, f32, tag=handle.name)
                nc.sync.dma_start(out=t[:], in_=handle[:])
                return t

            coliota = cpool.tile([P, 1, P], dtm, tag="coliota")
            nc.sync.dma_start(out=coliota[:], in_=coliota_p[:, None, :])
            ident16 = cpool.tile([P, P], dtm, tag="ident16")
            nc.sync.dma_start(out=ident16[:], in_=ident16_p[:])
            ident32 = load_const(ident32_p, [P, P])
            W1s = load_const(W1_p, [64, 128])
            W2s = load_const(W2_p, [128, 128])
            W3s = load_const(W3_p, [128, 64])
            W4s = load_const(W4_p, [64, 2])
            W1h = cpool.tile([64, 128], dtm, tag="W1h")
            nc.vector.tensor_copy(out=W1h[:], in_=W1s[:])
            vt = {nm: load_const(h, list(h.shape)) for nm, h in vecs.items()}
            dinv_nm = load_const(dinv_nm_p, [P, nb])

            slot_sb = slpool.tile([P, tott, 1], dtm)
            nc.sync.dma_start(out=slot_sb[:], in_=slot_p[:, :, None])
            idx_sb = slpool.tile([P, totc], i16, tag="idx_sb")
            nc.sync.dma_start(out=idx_sb[:], in_=idx_p[:])
            dinvr = slpool.tile([P, npc], dtm, tag="dinvr")
            nc.sync.dma_start(out=dinvr[:], in_=dinvr_p[:])
            # w row for the rank-1 shift term; ones row for the bias term
            wrow = slpool.tile([1, npc], dtm, tag="wrow")
            nc.sync.dma_start(out=wrow[:], in_=wrow_p[:])
            onesP = cpool.tile([1, P], dtm, tag="onesP")
            nc.vector.memset(onesP[:], 1.0)
            rt = {}
            for nm, h in rows.items():
                t32 = load_const(h, list(h.shape))
                t16 = cpool.tile(list(h.shape), dtm, tag=nm + "h")
                nc.vector.tensor_copy(out=t16[:], in_=t32[:])
                rt[nm] = t16
            own_sb = opool.tile([P, nb, DW], dtm, tag="own_sb")
            nc.vector.memset(own_sb[:], 0.0)

            def ck(name):
                if stop_after == name:
                    raise _Stop()

            # ---------- T1 = dinv * x_pad (own slice) -> AllGather ----------
            for b in range(nb):
                xb = bpool.tile([P, D1], f32, tag="xb")
                nc.sync.dma_start(out=xb[:], in_=x_own[b * P:(b + 1) * P, :])
                t1b = bpool.tile([P, D1], dtm, tag="t1b")
                nc.vector.tensor_tensor(
                    out=t1b[:], in0=xb[:],
                    in1=dinv_nm[:, b:b + 1].to_broadcast([P, D1]),
                    op=Alu.mult)
                nc.vector.tensor_copy(out=own_sb[:, b, :D1], in_=t1b[:])
                nc.sync.dma_start(out=Tc[1][b * P:(b + 1) * P, :D1], in_=t1b[:])
            nc.gpsimd.collective_compute(
                "AllGather", Alu.bypass, replica_groups=rg,
                ins=[Tc[1].ap().opt()], outs=[T[1].ap().opt()])

            # ---------- helpers ----------
            gq = [0]   # global gather counter: Tile locks DMASW lane (mod 8)
                       # to SWDGE queue, so queue must follow the same counter

            def message_pass(D, table, post_block, ps_acc):
                for bi, binfo in enumerate(batches):
                    bnt, blks = binfo["nt"], binfo["blks"]
                    if bnt == 0:
                        continue
                    msgs = {}
                    for ci, (k, ct0, cnt) in enumerate(binfo["calls"]):
                        nidx = cnt * P
                        msgs[ci] = mpool.tile([P, CALL_TILES, DW], dtm,
                                              tag="msg", name=f"msg{ci}")
                        nc.gpsimd.dma_gather(
                            out_ap=msgs[ci][:, :cnt, :],
                            in_ap=table[k * bs:(k + 1) * bs, :],
                            idxs_ap=idx_sb[:, ct0 * 8:(ct0 + cnt) * 8],
                            num_idxs=nidx, num_idxs_reg=nidx, elem_size=DW,
                            queue_num=gq[0] % 4)
                        gq[0] += 1
                    if MP_MODE == "gather":
                        continue
                    accs = {}
                    first = {b: False for b in blks}
                    last_mm = {b: None for b in blks}
                    for b in blks:
                        accs[b] = ps_acc.tile([DW, P], f32, tag="acc",
                                              name=f"acc{b}")
                        last_mm[b] = binfo["btiles"][b][-1][1] + \
                            binfo["btiles"][b][-1][2] - 1 if binfo["btiles"][b] else None
                        # self-loop: identity matmul of the block's own table
                        # rows (acc[:, j] += T[row j]), kept in SBUF
                        nc.tensor.matmul(out=accs[b][:], lhsT=own_sb[:, b, :],
                                         rhs=ident16[:],
                                         start=True, stop=(last_mm[b] is None))
                    for ci, (k, ct0, cnt) in enumerate(binfo["calls"]):
                        S = spool.tile([P, CALL_TILES, P], dtm, tag="S",
                                       name=f"S{ci}")
                        nc.vector.tensor_tensor(
                            out=S[:, :cnt, :],
                            in0=slot_sb[:, ct0:ct0 + cnt, :].to_broadcast([P, cnt, P]),
                            in1=coliota[:].to_broadcast([P, cnt, P]),
                            op=Alu.is_equal)
                        if MP_MODE == "smat":
                            continue
                        for b in blks:
                            for (kk, t0, nt) in binfo["btiles"][b]:
                                if kk != k:
                                    continue
                                lo = max(t0, ct0)
                                hi = min(t0 + nt, ct0 + cnt)
                                for t in range(lo, hi):
                                    nc.tensor.matmul(
                                        out=accs[b][:],
                                        lhsT=msgs[ci][:, t - ct0, :],
                                        rhs=S[:, t - ct0, :],
                                        start=first[b], stop=(t == last_mm[b]))
                                    first[b] = False
                    if MP_MODE in ("smat", "mm"):
                        continue
                    for b in blks:
                        post_block(b, accs[b])

            def dinv_b(b, F):
                return dinvr[:F, b * P:(b + 1) * P]

            def w_b(b):
                return wrow[0:1, b * P:(b + 1) * P]

            def bn_phase(layer, F, ssum, ssq, cs, cq):
                """stats -> AllReduce -> (a, c) affine for BN `layer`."""
                stat = stpool.tile([F, 2], f32, tag=f"stat{layer}")
                nc.vector.tensor_reduce(out=stat[:, 0:1], in_=ssum[:],
                                        axis=Axis.X, op=Alu.add)
                nc.vector.tensor_reduce(out=stat[:, 1:2], in_=ssq[:],
                                        axis=Axis.X, op=Alu.add)
                nc.vector.tensor_tensor(out=stat[:, 0:1], in0=stat[:, 0:1],
                                        in1=cs[:], op=Alu.add)
                nc.vector.tensor_tensor(out=stat[:, 1:2], in0=stat[:, 1:2],
                                        in1=cq[:], op=Alu.add)
                nc.sync.dma_start(out=st_in[layer][:], in_=stat[:])
                nc.gpsimd.collective_compute(
                    "AllReduce", Alu.add, replica_groups=rg,
                    ins=[st_in[layer].ap().opt()], outs=[st_out[layer].ap().opt()])
                stg = stpool.tile([F, 2], f32, tag=f"statg{layer}")
                nc.sync.dma_start(out=stg[:], in_=st_out[layer][:])
                mean = stpool.tile([F, 1], f32, tag=f"mean{layer}")
                nc.vector.tensor_scalar(out=mean[:], in0=stg[:, 0:1],
                                        scalar1=1.0 / N, scalar2=None, op0=Alu.mult)
                var = stpool.tile([F, 1], f32, tag=f"var{layer}")
                nc.vector.tensor_scalar(out=var[:], in0=stg[:, 1:2],
                                        scalar1=1.0 / N, scalar2=None, op0=Alu.mult)
                msq = stpool.tile([F, 1], f32, tag=f"msq{layer}")
                nc.vector.tensor_tensor(out=msq[:], in0=mean[:], in1=mean[:],
                                        op=Alu.mult)
                nc.vector.tensor_tensor(out=var[:], in0=var[:], in1=msq[:],
                                        op=Alu.subtract)
                nc.vector.tensor_scalar(out=var[:], in0=var[:], scalar1=0.0,
                                        scalar2=None, op0=Alu.max)
                ve = stpool.tile([F, 1], f32, tag=f"ve{layer}")
                nc.vector.tensor_scalar(out=ve[:], in0=var[:], scalar1=1e-5,
                                        scalar2=None, op0=Alu.add)
                sd = stpool.tile([F, 1], f32, tag=f"sd{layer}")
                nc.scalar.activation(out=sd[:], in_=ve[:], func=Act.Sqrt)
                inv = stpool.tile([F, 1], f32, tag=f"inv{layer}")
                nc.vector.reciprocal(out=inv[:], in_=sd[:])
                a = stpool.tile([F, 1], f32, tag=f"a{layer}")
                nc.vector.tensor_tensor(out=a[:], in0=vt[f"g{layer}"][:],
                                        in1=inv[:], op=Alu.mult)
                am = stpool.tile([F, 1], f32, tag=f"am{layer}")
                nc.vector.tensor_tensor(out=am[:], in0=a[:], in1=mean[:],
                                        op=Alu.mult)
                cc = stpool.tile([F, 1], f32, tag=f"bb{layer}")
                nc.vector.tensor_tensor(out=cc[:], in0=vt[f"be{layer}"][:],
                                        in1=am[:], op=Alu.subtract)
                if DEBUG_DUMP:
                    nc.sync.dma_start(out=dbg[f"dst{layer}"][:], in_=stg[:])
                    ac = stpool.tile([F, 2], f32, tag=f"dac{layer}")
                    nc.vector.tensor_copy(out=ac[:, 0:1], in_=a[:])
                    nc.vector.tensor_copy(out=ac[:, 1:2], in_=cc[:])
                    nc.sync.dma_start(out=dbg[f"dac{layer}"][:], in_=ac[:])
                return a, cc

            def weight_prep(layer, Fin, Fout, Wn, a, c, ps):
                """Fold BN_{layer-1} affine into W_layer:
                   W'' = diag(a) W (f16), crow = c @ W  [1, Fout]."""
                Wpp = stpool.tile([Fin, Fout], dtm, tag=f"Wpp{layer}")
                nc.vector.tensor_tensor(out=Wpp[:], in0=Wn[:],
                                        in1=a[:].to_broadcast([Fin, Fout]),
                                        op=Alu.mult)
                crow_ps = ps.tile([1, Fout], f32, tag="crow")
                nc.tensor.matmul(out=crow_ps[:], lhsT=c[:], rhs=Wn[:],
                                 start=True, stop=True)
                crow = stpool.tile([1, Fout], dtm, tag=f"crow{layer}")
                nc.vector.tensor_copy(out=crow[:], in_=crow_ps[:])
                return Wpp, crow

            def make_post(layer, Fin, Fout, ssum, ssq, Wpp, crow, brow,
                          tcn, ps_aux, logits=None):
                """post for layers 2..4: acc -> dinv scale -> GEMM + rank-1
                -> relu(+bias) -> stats -> next-layer table rows."""
                def post(b, acc):
                    sacc = bpool.tile([Fin, P], dtm, tag="sacc")
                    nc.vector.tensor_tensor(out=sacc[:], in0=acc[:Fin, :],
                                            in1=dinv_b(b, Fin), op=Alu.mult)
                    if layer == 4:
                        lgt = ps_aux.tile([P, P], f32, tag="aux")
                        lg = lgt[:, :2]
                        nc.tensor.matmul(out=lg, lhsT=sacc[:], rhs=Wpp[:],
                                         start=True, stop=False)
                        nc.tensor.matmul(out=lg, lhsT=w_b(b), rhs=crow[:],
                                         start=False, stop=False)
                        nc.tensor.matmul(out=lg, lhsT=onesP[:], rhs=brow[:],
                                         start=False, stop=True)
                        nc.vector.tensor_copy(out=logits[:, b, :], in_=lg)
                        return
                    zt = ps_aux.tile([P, P], f32, tag="aux")
                    z = zt[:Fout, :]
                    nc.tensor.matmul(out=z, lhsT=Wpp[:], rhs=sacc[:],
                                     start=True, stop=False)
                    nc.tensor.matmul(out=z, lhsT=crow[:], rhs=w_b(b),
                                     start=False, stop=False)
                    nc.tensor.matmul(out=z, lhsT=brow[:], rhs=onesP[:],
                                     start=False, stop=True)
                    r = bpool.tile([Fout, P], dtm, tag="rblk")
                    nc.scalar.activation(out=r[:], in_=z, func=Act.Relu,
                                         accum_out=ssum[:, b:b + 1])
                    sq = bpool.tile([Fout, P], f32, tag="sq")
                    nc.scalar.activation(out=sq[:], in_=r[:], func=Act.Square,
                                         accum_out=ssq[:, b:b + 1])
                    tn = bpool.tile([Fout, P], f32, tag="tnext")
                    nc.vector.tensor_tensor(out=tn[:], in0=r[:],
                                            in1=dinv_b(b, Fout), op=Alu.mult)
                    ptt = ps_aux.tile([P, P], f32, tag="aux")
                    pt = ptt[:, :Fout]
                    nc.tensor.transpose(out=pt, in_=tn[:],
                                        identity=ident32[:Fout, :Fout])
                    nc.vector.tensor_copy(out=own_sb[:, b, :Fout], in_=pt)
                    nc.sync.dma_start(out=tcn[b * P:(b + 1) * P, :Fout],
                                      in_=own_sb[:, b, :Fout])
                return post

            for _rep in range(repeat):
                try:
                    # ======== layer 1 ========
                    ck("setup")
                    ssum1 = stpool.tile([F1, nb], f32, tag="ssum1")
                    ssq1 = stpool.tile([F1, nb], f32, tag="ssq1")
                    with tc.tile_pool(name="ps1", bufs=6, space="PSUM") as ps_acc, \
                         tc.tile_pool(name="ps1x", bufs=2, space="PSUM") as ps_aux:

                        def post1(b, acc):
                            s1 = bpool.tile([64, P], dtm, tag="sacc")
                            nc.vector.tensor_tensor(
                                out=s1[:], in0=acc[:64, :],
                                in1=dinv_b(b, 64), op=Alu.mult)
                            zt = ps_aux.tile([P, P], f32, tag="aux")
                            z = zt[:128, :]
                            nc.tensor.matmul(out=z, lhsT=W1h[:], rhs=s1[:],
                                             start=True, stop=True)
                            r = bpool.tile([F1, P], dtm, tag="rblk")
                            nc.scalar.activation(out=r[:], in_=z,
                                                 func=Act.Relu,
                                                 bias=vt["b1"][:], scale=1.0,
                                                 accum_out=ssum1[:, b:b + 1])
                            sq = bpool.tile([F1, P], f32, tag="sq")
                            nc.scalar.activation(out=sq[:], in_=r[:],
                                                 func=Act.Square,
                                                 accum_out=ssq1[:, b:b + 1])
                            tn = bpool.tile([F1, P], f32, tag="tnext")
                            nc.vector.tensor_tensor(out=tn[:], in0=r[:],
                                                    in1=dinv_b(b, F1),
                                                    op=Alu.mult)
                            ptt = ps_aux.tile([P, P], f32, tag="aux")
                            pt = ptt[:, :F1]
                            nc.tensor.transpose(out=pt, in_=tn[:],
                                                identity=ident32[:F1, :F1])
                            nc.vector.tensor_copy(out=own_sb[:, b, :F1],
                                                  in_=pt)
                            nc.sync.dma_start(out=Tc[2][b * P:(b + 1) * P, :F1],
                                              in_=own_sb[:, b, :F1])

                        message_pass(D1, T[1], post1, ps_acc)
                    ck("mp1")
                    nc.gpsimd.collective_compute(
                        "AllGather", Alu.bypass, replica_groups=rg,
                        ins=[Tc[2].ap().opt()], outs=[T[2].ap().opt()])
                    if DEBUG_DUMP:
                        nc.sync.dma_start(out=dbg["dT2"][:], in_=Tc[2][:])
                    a1, c1 = bn_phase(1, F1, ssum1, ssq1, vt["cs1"], vt["cq1"])
                    with tc.tile_pool(name="pw1", bufs=2, space="PSUM") as psw:
                        W2pp, crow2 = weight_prep(2, F1, F2, W2s, a1, c1, psw)
                    ck("bn1")

                    # ======== layer 2 ========
                    ssum2 = stpool.tile([F2, nb], f32, tag="ssum2")
                    ssq2 = stpool.tile([F2, nb], f32, tag="ssq2")
                    with tc.tile_pool(name="ps2", bufs=6, space="PSUM") as ps_acc, \
                         tc.tile_pool(name="ps2x", bufs=2, space="PSUM") as ps_aux:
                        post2 = make_post(2, D2, F2, ssum2, ssq2, W2pp,
                                          crow2, rt["b2r"], Tc[3], ps_aux)
                        message_pass(D2, T[2], post2, ps_acc)
                    ck("mp2")
                    nc.gpsimd.collective_compute(
                        "AllGather", Alu.bypass, replica_groups=rg,
                        ins=[Tc[3].ap().opt()], outs=[T[3].ap().opt()])
                    if DEBUG_DUMP:
                        nc.sync.dma_start(out=dbg["dT3"][:], in_=Tc[3][:])
                    a2, c2 = bn_phase(2, F2, ssum2, ssq2,
                                      vt["cs2"], vt["cq2"])
                    with tc.tile_pool(name="pw2", bufs=2, space="PSUM") as psw:
                        W3pp, crow3 = weight_prep(3, F2, F3, W3s, a2, c2, psw)
                    ck("bn2")

                    # ======== layer 3 ========
                    ssum3 = stpool.tile([F3, nb], f32, tag="ssum3")
                    ssq3 = stpool.tile([F3, nb], f32, tag="ssq3")
                    with tc.tile_pool(name="ps3", bufs=6, space="PSUM") as ps_acc, \
                         tc.tile_pool(name="ps3x", bufs=2, space="PSUM") as ps_aux:
                        post3 = make_post(3, D3, F3, ssum3, ssq3, W3pp,
                                          crow3, rt["b3r"], Tc[4], ps_aux)
                        message_pass(D3, T[3], post3, ps_acc)
                    ck("mp3")
                    nc.gpsimd.collective_compute(
                        "AllGather", Alu.bypass, replica_groups=rg,
                        ins=[Tc[4].ap().opt()], outs=[T[4].ap().opt()])
                    if DEBUG_DUMP:
                        nc.sync.dma_start(out=dbg["dT4"][:], in_=Tc[4][:])
                    a3, c3 = bn_phase(3, F3, ssum3, ssq3,
                                      vt["cs3"], vt["cq3"])
                    with tc.tile_pool(name="pw3", bufs=2, space="PSUM") as psw:
                        W4pp, crow4 = weight_prep(4, F3, F4, W4s, a3, c3, psw)
                    ck("bn3")

                    # ======== layer 4 ========
                    logits = stpool.tile([P, nb, 2], f32, tag="logits")
                    with tc.tile_pool(name="ps4", bufs=6, space="PSUM") as ps_acc, \
                         tc.tile_pool(name="ps4x", bufs=2, space="PSUM") as ps_aux:
                        post4 = make_post(4, D4, F4, None, None, W4pp,
                                          crow4, rt["b4r"], None, ps_aux,
                                          logits=logits)
                        message_pass(D4, T[4], post4, ps_acc)
                    ck("mp4")

                    mx = stpool.tile([P, nb, 1], f32, tag="mx")
                    nc.vector.tensor_reduce(out=mx[:], in_=logits[:],
                                            axis=Axis.X, op=Alu.max)
                    dz = stpool.tile([P, nb, 2], f32, tag="dz")
                    nc.vector.tensor_tensor(out=dz[:], in0=logits[:],
                                            in1=mx[:].to_broadcast([P, nb, 2]),
                                            op=Alu.subtract)
                    ez = stpool.tile([P, nb, 2], f32, tag="ez")
                    nc.scalar.activation(out=ez[:], in_=dz[:], func=Act.Exp)
                    se = stpool.tile([P, nb, 1], f32, tag="se")
                    nc.vector.tensor_reduce(out=se[:], in_=ez[:], axis=Axis.X,
                                            op=Alu.add)
                    ls = stpool.tile([P, nb, 1], f32, tag="ls")
                    nc.scalar.activation(out=ls[:], in_=se[:], func=Act.Ln)
                    ov = stpool.tile([P, nb, 2], f32, tag="ov")
                    nc.vector.tensor_tensor(out=ov[:], in0=dz[:],
                                            in1=ls[:].to_broadcast([P, nb, 2]),
                                            op=Alu.subtract)
                    nc.sync.dma_start(out=out_p[:], in_=ov[:])
                except _Stop:
                    zz = stpool.tile([P, nb, 2], f32, tag="zz")
                    nc.vector.memset(zz[:], 0.0)
                    nc.sync.dma_start(out=out_p[:], in_=zz[:])

    nc.compile()
    return nc


# ------------------------------------------------------------------ driver

_CACHE = {}


def _prep_in_maps(plan, inputs):
    n_cores, npc, nb = plan["n_cores"], plan["npc"], plan["nb"]
    N = plan["N"]
    dinv = plan["dinv"]
    w = plan["w"]

    x = np.asarray(inputs["x"], np.float32)
    xpad = np.zeros((plan["npad"], 64), np.float32)
    xpad[plan["newid"][:N], :x.shape[1]] = x

    W1 = np.asarray(inputs["W1"], np.float32)
    W1p = np.zeros((64, 128), np.float32)
    W1p[:W1.shape[0]] = W1
    dt_m = np.float16 if F16 else np.float32
    coliota = np.broadcast_to(np.arange(P, dtype=dt_m), (P, P)).copy()

    def col(v):
        return np.ascontiguousarray(np.asarray(v, np.float32).reshape(-1, 1))

    b1 = np.asarray(inputs["b1"], np.float32)
    rb1 = np.maximum(b1, 0.0)
    rb2 = np.maximum(np.asarray(inputs["b2"], np.float32), 0.0)
    rb3 = np.maximum(np.asarray(inputs["b3"], np.float32), 0.0)
    K = plan["npad"] - N
    common = {
        "W1p": W1p, "W2": np.asarray(inputs["W2"], np.float32),
        "W3": np.asarray(inputs["W3"], np.float32),
        "W4": np.asarray(inputs["W4"], np.float32),
        "coliota": coliota,
        "ident16": np.eye(P, dtype=dt_m),
        "ident32": np.eye(P, dtype=np.float32),
        "b2r": np.asarray(inputs["b2"], np.float32).reshape(1, -1),
        "b3r": np.asarray(inputs["b3"], np.float32).reshape(1, -1),
        "b4r": np.asarray(inputs["b4"], np.float32).reshape(1, -1),
        "b1": col(b1), "g1": col(inputs["g1"]), "be1": col(inputs["be1"]),
        "cs1": col(-K * rb1 / n_cores),
        "cq1": col(-K * rb1 * rb1 / n_cores),
        "cs2": col(-K * rb2 / n_cores),
        "cq2": col(-K * rb2 * rb2 / n_cores),
        "cs3": col(-K * rb3 / n_cores),
        "cq3": col(-K * rb3 * rb3 / n_cores),
        "b2": col(inputs["b2"]), "g2": col(inputs["g2"]),
        "be2": col(inputs["be2"]),
        "b3": col(inputs["b3"]), "g3": col(inputs["g3"]),
        "be3": col(inputs["be3"]),
        "b4c": col(inputs["b4"]),
    }

    in_maps = []
    for c in range(n_cores):
        dc = dinv[c * npc:(c + 1) * npc]
        wc = w[c * npc:(c + 1) * npc]
        m = dict(common)
        m["x_own"] = np.ascontiguousarray(xpad[c * npc:(c + 1) * npc])
        m["idx"] = plan["idx_arrs"][c]
        m["slot"] = plan["slot_arrs"][c].astype(dt_m)
        m["dinvr"] = np.ascontiguousarray(
            np.broadcast_to(dc, (P, npc)).astype(dt_m))
        m["wrow"] = np.ascontiguousarray(
            wc.reshape(1, -1).astype(dt_m))
        m["dinv_nm"] = np.ascontiguousarray(dc.reshape(nb, P).T)
        in_maps.append(m)
    return in_maps


def _unshard(plan, outs):
    nb, npc, N = plan["nb"], plan["npc"], plan["N"]
    parts = []
    for c in range(plan["n_cores"]):
        o = np.asarray(outs[c]["out"]).reshape(P, nb, 2)
        parts.append(np.ascontiguousarray(o.transpose(1, 0, 2)).reshape(npc, 2))
    full = np.concatenate(parts, axis=0)
    return full[plan["newid"][:N]]


LAST_EXEC_NS = None


def kernel(**inputs):
    global LAST_EXEC_NS
    from concourse.bass_utils import run_bass_kernel_spmd

    edge_index = np.asarray(inputs["edge_index"])
    N = int(np.asarray(inputs["x"]).shape[0])
    plan = _build_plan(edge_index, N, 8)

    key = ("v11", N, plan["tott"], F16,
           tuple((b["t0"], b["nt"]) for b in plan["batches"]),
           tuple(c for b in plan["batches"] for c in b["calls"]))
    if key not in _CACHE:
        _CACHE[key] = _build_nc(plan)
    nc = _CACHE[key]

    in_maps = _prep_in_maps(plan, inputs)
    trace = bool(int(os.environ.get("GCN_TRACE", "0")))
    res = run_bass_kernel_spmd(nc, in_maps, list(range(8)), trace=trace)
    LAST_EXEC_NS = res.exec_time_ns
    return _unshard(plan, res.results)
